# revision 2
# baseline (speedup 1.0000x reference)
"""NodeConv kernel for 8 Trainium2 NeuronCores.

Reference computes, for adj [B,1,N,N], node [B,nin,N], Wi/Wj [nout,nin]:
    x  = node[:, :, None, :] * adj          # [B,nin,N,N]
    yi = einsum('oc,bcij->boij', Wi, x)
    yj = einsum('oc,bcij->boij', Wj, x)
    out = I * yi + (1-I) * yj

Because adj[b,i,j] does not depend on the contraction channel c, the
contraction factors out:
    off-diag: out[b,o,i,j] = adj[b,i,j] * (Wj @ node[b])[o,j]
    diag:     out[b,o,j,j] = adj[b,j,j] * (Wi @ node[b])[o,j]

So per batch we need two tiny 128x128 @ 128x256 matmuls (u = Wj@node,
v = Wi@node) and then a broadcast multiply out[o,i,j] = adj[i,j]*u[o,j]
with a diagonal patch.  The 128 MiB output write is the roofline.

Sharding: 8 cores; core c handles batch b=c//2, row half h=c%2 (128 rows).
To keep one SPMD program, odd halves get their columns rolled by -128 on
the host so the diagonal element of local row l always sits at local
column l; the host rolls the output back when gathering.

Per-core device program:
  - u = Wj @ node_r (PE), v = Wi @ node_r[:, :128] (PE)
  - dv[o,l] = adj_diag[l] * v[o,l]   (PE one-hot broadcast + DVE mult)
  - u_rep = u tiled 8x along free dim -> [128, 2048]
  - for each chunk p of 8 rows: PE broadcasts the 8 adj rows to all 128
    partitions via a one-hot selector matmul (K=16, rhs = adj chunk held
    on partition p), DVE multiplies by u_rep, a strided (stride 257)
    copy patches the 8 diagonal elements, DMA stores 1 MiB.
"""

import os

import numpy as np

NCORES = 8
B, N, NIN, NOUT = 4, 256, 128, 128
RPC = 128          # rows per core
CH = 16            # chunks per core
RCH = 8            # rows per chunk
FREE = RCH * N     # 2048 free elems per chunk

# matmul dtype for the broadcast matmuls: "float32" (exact, 4 cyc/row) or
# "float32r" (1 cyc/row at free>=256; verify numerics on HW first).
BCAST_DT = os.environ.get("NODECONV_BCAST_DT", "float32")

_cached = {}

last_results = None  # BassKernelResults of the most recent kernel() call


def _build_nc():
    key = BCAST_DT
    if key in _cached:
        return _cached[key]

    from contextlib import ExitStack

    import concourse.tile as tile
    from concourse import bacc, mybir

    f32 = mybir.dt.float32
    mmdt = getattr(mybir.dt, BCAST_DT)

    nc = bacc.Bacc(
        "TRN2", target_bir_lowering=False, debug=False, num_devices=NCORES
    )

    adjf = nc.dram_tensor("adjf", [CH, FREE], f32, kind="ExternalInput").ap()
    node_r = nc.dram_tensor("node_r", [NIN, N], f32, kind="ExternalInput").ap()
    diag16 = nc.dram_tensor("diag16", [CH, RPC], f32, kind="ExternalInput").ap()
    wit = nc.dram_tensor("wit", [NIN, NOUT], f32, kind="ExternalInput").ap()
    wjt = nc.dram_tensor("wjt", [NIN, NOUT], f32, kind="ExternalInput").ap()
    sel = nc.dram_tensor("sel", [CH, CH * NOUT], f32, kind="ExternalInput").ap()
    out = nc.dram_tensor("out", [NOUT, RPC * N], f32, kind="ExternalOutput").ap()

    def mm(ap):
        return ap.bitcast(mmdt) if mmdt != f32 else ap

    with tile.TileContext(nc) as tc, ExitStack() as ctx:
        const = ctx.enter_context(tc.tile_pool(name="const", bufs=1))
        psum = ctx.enter_context(tc.tile_pool(name="psum", bufs=2, space="PSUM"))
        outp = ctx.enter_context(tc.tile_pool(name="outp", bufs=3))

        adjf_sb = const.tile([CH, FREE], f32)
        nc.sync.dma_start(out=adjf_sb[:], in_=adjf)
        node_sb = const.tile([NIN, N], f32)
        nc.sync.dma_start(out=node_sb[:], in_=node_r)
        diag_sb = const.tile([CH, RPC], f32)
        nc.sync.dma_start(out=diag_sb[:], in_=diag16)
        wit_sb = const.tile([NIN, NOUT], f32)
        nc.sync.dma_start(out=wit_sb[:], in_=wit)
        wjt_sb = const.tile([NIN, NOUT], f32)
        nc.sync.dma_start(out=wjt_sb[:], in_=wjt)
        sel_sb = const.tile([CH, CH * NOUT], f32)
        nc.sync.dma_start(out=sel_sb[:], in_=sel)

        # u = Wj @ node_r  -> [nout, N]
        ps_u = psum.tile([NOUT, N], f32, tag="mm")
        nc.tensor.matmul(ps_u[:], lhsT=wjt_sb[:], rhs=node_sb[:], start=True, stop=True)
        u_sb = const.tile([NOUT, N], f32)
        nc.scalar.copy(u_sb[:], ps_u[:])

        # v = Wi @ node_r[:, :128]  (only the diagonal columns are needed)
        ps_v = psum.tile([NOUT, RPC], f32, tag="mm")
        nc.tensor.matmul(
            ps_v[:], lhsT=wit_sb[:], rhs=node_sb[:, 0:RPC], start=True, stop=True
        )
        v_sb = const.tile([NOUT, RPC], f32)
        nc.scalar.copy(v_sb[:], ps_v[:])

        # dv[o,l] = adj_diag[l] * v[o,l]; broadcast diag row over partitions
        ps_d = psum.tile([NOUT, RPC], f32, tag="mm")
        nc.tensor.matmul(
            ps_d[:], lhsT=sel_sb[:, 0:NOUT], rhs=diag_sb[:], start=True, stop=True
        )
        dv_sb = const.tile([NOUT, RPC], f32)
        nc.vector.tensor_mul(dv_sb[:], ps_d[:], v_sb[:])

        # u_rep: u tiled RCH times along free dim
        u_rep = const.tile([NOUT, FREE], f32)
        nc.scalar.copy(u_rep[:, 0:N], u_sb[:])
        rep = N
        while rep < FREE:
            nc.scalar.copy(u_rep[:, rep : 2 * rep], u_rep[:, 0:rep])
            rep *= 2

        for p in range(CH):
            ps_b = psum.tile([NOUT, FREE], f32, tag="mm")
            for q in range(FREE // 512):
                nc.tensor.matmul(
                    ps_b[:, 512 * q : 512 * (q + 1)],
                    lhsT=mm(sel_sb[:, NOUT * p : NOUT * (p + 1)]),
                    rhs=mm(adjf_sb[:, 512 * q : 512 * (q + 1)]),
                    start=True,
                    stop=True,
                )
            o_sb = outp.tile([NOUT, FREE], f32)
            nc.vector.tensor_mul(o_sb[:], ps_b[:], u_rep[:])
            # diagonal of local row l=8p+k sits at free offset 8p + k*257
            nc.vector.tensor_copy(
                o_sb[:, RCH * p : RCH * p + (RCH - 1) * (N + 1) + 1 : N + 1],
                dv_sb[:, RCH * p : RCH * (p + 1)],
            )
            nc.sync.dma_start(out=out[:, FREE * p : FREE * (p + 1)], in_=o_sb[:])

    nc.compile()
    _cached[key] = nc
    return nc


def _in_maps(adj, node, Wi, Wj):
    sel = np.zeros((CH, CH * NOUT), np.float32)
    for p in range(CH):
        sel[p, NOUT * p : NOUT * (p + 1)] = 1.0
    wit = np.ascontiguousarray(Wi.T)
    wjt = np.ascontiguousarray(Wj.T)
    maps = []
    for c in range(NCORES):
        b, h = divmod(c, 2)
        r0 = RPC * h
        a = adj[b, 0, r0 : r0 + RPC, :]
        diag = np.zeros((CH, RPC), np.float32)
        diag[0, :] = a[np.arange(RPC), r0 + np.arange(RPC)]
        if h:
            ar = np.roll(a, -r0, axis=1)
            noder = np.roll(node[b], -r0, axis=1)
        else:
            ar = a
            noder = node[b]
        maps.append(
            {
                "adjf": np.ascontiguousarray(ar.reshape(CH, FREE)),
                "node_r": np.ascontiguousarray(noder),
                "diag16": diag,
                "wit": wit,
                "wjt": wjt,
                "sel": sel,
            }
        )
    return maps


def kernel(**inputs):
    global last_results
    adj = np.asarray(inputs["adj"], dtype=np.float32)
    node = np.asarray(inputs["node"], dtype=np.float32)
    Wi = np.asarray(inputs["Wi"], dtype=np.float32)
    Wj = np.asarray(inputs["Wj"], dtype=np.float32)

    from concourse.bass_utils import run_bass_kernel_spmd

    nc = _build_nc()
    res = run_bass_kernel_spmd(nc, _in_maps(adj, node, Wi, Wj), list(range(NCORES)))
    last_results = res

    out = np.empty((B, NOUT, N, N), np.float32)
    for c in range(NCORES):
        b, h = divmod(c, 2)
        co = res.results[c]["out"].reshape(NOUT, RPC, N)
        if h:
            co = np.roll(co, RPC * h, axis=2)
        out[b, :, RPC * h : RPC * (h + 1), :] = co
    return out


# revision 7
# speedup vs baseline: 1.6477x; 1.6477x over previous
"""NodeConv kernel for 8 Trainium2 NeuronCores.

Reference computes, for adj [B,1,N,N], node [B,nin,N], Wi/Wj [nout,nin]:
    x  = node[:, :, None, :] * adj          # [B,nin,N,N]
    yi = einsum('oc,bcij->boij', Wi, x)
    yj = einsum('oc,bcij->boij', Wj, x)
    out = I * yi + (1-I) * yj

Because adj[b,i,j] does not depend on the contraction channel c, the
contraction factors out:
    off-diag: out[b,o,i,j] = adj[b,i,j] * (Wj @ node[b])[o,j]
    diag:     out[b,o,j,j] = adj[b,j,j] * (Wi @ node[b])[o,j]

So per batch we need two tiny 128x128 @ 128x256 matmuls (u = Wj@node,
v = Wi@node) and then a broadcast multiply out[o,i,j] = adj[i,j]*u[o,j]
with a diagonal patch.  The 128 MiB output write is the roofline.

Sharding: 8 cores; core c handles batch b=c//2, row half h=c%2 (128 rows).
To keep one SPMD program, odd halves get their columns rolled by -128 on
the host so the diagonal element of local row l always sits at local
column l; the host rolls the output back when gathering.

Per-core device program:
  - u = Wj @ node_r (PE), v = Wi @ node_r[:, :128] (PE)
  - dv[o,l] = adj_diag[l] * v[o,l]   (PE one-hot broadcast + DVE mult)
  - u_rep = u tiled 8x along free dim -> [128, 2048]
  - for each chunk p of 8 rows: PE broadcasts the 8 adj rows to all 128
    partitions via a one-hot selector matmul (K=16, rhs = adj chunk held
    on partition p), DVE multiplies by u_rep, a strided (stride 257)
    copy patches the 8 diagonal elements, DMA stores 1 MiB.
"""

import os

import numpy as np

NCORES = 8
B, N, NIN, NOUT = 4, 256, 128, 128
RPC = 128          # rows per core
CH = 16            # chunks per core
RCH = 8            # rows per chunk
FREE = RCH * N     # 2048 free elems per chunk

# Broadcast-matmul mode: "float32" (exact, 4 cyc/row — PE-bound) or
# "bf16x2"/"bf16x3" (adj split into 2/3 bf16 terms accumulated in PSUM;
# bf16 streams 1 cyc/row.  x2 leaves ~2^-18 relative rounding on adj,
# x3 reconstructs fp32 exactly).
BCAST_DT = os.environ.get("NODECONV_BCAST_DT", "bf16x2")

_cached = {}

last_results = None  # BassKernelResults of the most recent kernel() call


def _build_nc():
    key = BCAST_DT
    if key in _cached:
        return _cached[key]

    from contextlib import ExitStack

    import concourse.tile as tile
    from concourse import bacc, mybir

    f32 = mybir.dt.float32
    bf16 = mybir.dt.bfloat16
    nterms = {"float32": 0, "bf16x2": 2, "bf16x3": 3}[BCAST_DT]

    nc = bacc.Bacc(
        "TRN2", target_bir_lowering=False, debug=False, num_devices=NCORES
    )

    adt = f32 if nterms == 0 else bf16
    adj_in = [
        nc.dram_tensor(f"adj{t}", [CH, FREE], adt, kind="ExternalInput").ap()
        for t in range(max(nterms, 1))
    ]
    node_r = nc.dram_tensor("node_r", [NIN, N], f32, kind="ExternalInput").ap()
    diag16 = nc.dram_tensor("diag16", [CH, RPC], f32, kind="ExternalInput").ap()
    wit = nc.dram_tensor("wit", [NIN, NOUT], f32, kind="ExternalInput").ap()
    wjt = nc.dram_tensor("wjt", [NIN, NOUT], f32, kind="ExternalInput").ap()
    sel = nc.dram_tensor("sel", [CH, CH * NOUT], adt, kind="ExternalInput").ap()
    selz = nc.dram_tensor("selz", [CH, NOUT], f32, kind="ExternalInput").ap()
    out = nc.dram_tensor("out", [NOUT, RPC * N], f32, kind="ExternalOutput").ap()

    with tile.TileContext(nc) as tc, ExitStack() as ctx:
        const = ctx.enter_context(tc.tile_pool(name="const", bufs=1))
        psum = ctx.enter_context(tc.tile_pool(name="psum", bufs=2, space="PSUM"))
        outp = ctx.enter_context(tc.tile_pool(name="outp", bufs=3))

        adj_sb = []
        for t, ap_in in enumerate(adj_in):
            sb = const.tile([CH, FREE], adt, tag=f"adj{t}")
            nc.sync.dma_start(out=sb[:], in_=ap_in)
            adj_sb.append(sb)
        node_sb = const.tile([NIN, N], f32)
        nc.sync.dma_start(out=node_sb[:], in_=node_r)
        diag_sb = const.tile([CH, RPC], f32)
        nc.sync.dma_start(out=diag_sb[:], in_=diag16)
        wit_sb = const.tile([NIN, NOUT], f32)
        nc.sync.dma_start(out=wit_sb[:], in_=wit)
        wjt_sb = const.tile([NIN, NOUT], f32)
        nc.sync.dma_start(out=wjt_sb[:], in_=wjt)
        sel_sb = const.tile([CH, CH * NOUT], adt)
        nc.sync.dma_start(out=sel_sb[:], in_=sel)
        selz_sb = const.tile([CH, NOUT], f32)
        nc.sync.dma_start(out=selz_sb[:], in_=selz)

        # u = Wj @ node_r  -> [nout, N]
        ps_u = psum.tile([NOUT, N], f32, tag="mm")
        nc.tensor.matmul(ps_u[:], lhsT=wjt_sb[:], rhs=node_sb[:], start=True, stop=True)
        u_sb = const.tile([NOUT, N], f32)
        nc.scalar.copy(u_sb[:], ps_u[:])

        # v = Wi @ node_r[:, :128]  (only the diagonal columns are needed)
        ps_v = psum.tile([NOUT, RPC], f32, tag="mm")
        nc.tensor.matmul(
            ps_v[:], lhsT=wit_sb[:], rhs=node_sb[:, 0:RPC], start=True, stop=True
        )
        v_sb = const.tile([NOUT, RPC], f32)
        nc.scalar.copy(v_sb[:], ps_v[:])

        # dv[o,l] = adj_diag[l] * v[o,l]; broadcast diag row over partitions
        ps_d = psum.tile([NOUT, RPC], f32, tag="mm")
        nc.tensor.matmul(
            ps_d[:], lhsT=selz_sb[:], rhs=diag_sb[:], start=True, stop=True
        )
        dv_sb = const.tile([NOUT, RPC], f32)
        nc.vector.tensor_mul(dv_sb[:], ps_d[:], v_sb[:])

        # u_rep: u tiled RCH times along free dim
        u_rep = const.tile([NOUT, FREE], f32)
        nc.scalar.copy(u_rep[:, 0:N], u_sb[:])
        rep = N
        while rep < FREE:
            nc.scalar.copy(u_rep[:, rep : 2 * rep], u_rep[:, 0:rep])
            rep *= 2

        for p in range(CH):
            ps_b = psum.tile([NOUT, FREE], f32, tag="mm")
            lhs = sel_sb[:, NOUT * p : NOUT * (p + 1)]
            for q in range(FREE // 512):
                sl = slice(512 * q, 512 * (q + 1))
                for t in range(len(adj_sb)):
                    nc.tensor.matmul(
                        ps_b[:, sl],
                        lhsT=lhs,
                        rhs=adj_sb[t][:, sl],
                        start=(t == 0),
                        stop=(t == len(adj_sb) - 1),
                    )
            o_sb = outp.tile([NOUT, FREE], f32)
            nc.vector.tensor_mul(o_sb[:], ps_b[:], u_rep[:])
            # diagonal of local row l=8p+k sits at free offset 8p + k*257
            nc.scalar.copy(
                o_sb[:, RCH * p : RCH * p + (RCH - 1) * (N + 1) + 1 : N + 1],
                dv_sb[:, RCH * p : RCH * (p + 1)],
            )
            nc.sync.dma_start(out=out[:, FREE * p : FREE * (p + 1)], in_=o_sb[:])

    nc.compile()
    _cached[key] = nc
    return nc


def _split_terms(x, nterms):
    """Split fp32 array into bf16 terms whose (fp32-)sum approximates x.
    2 terms leave <=2^-18 relative error; 3 terms are exact."""
    import ml_dtypes

    bf16 = ml_dtypes.bfloat16
    if nterms == 0:
        return [np.ascontiguousarray(x)]
    terms = []
    r = x
    for _ in range(nterms):
        t = r.astype(bf16)
        terms.append(np.ascontiguousarray(t))
        r = (r - t.astype(np.float32)).astype(np.float32)
    return terms


def _in_maps(adj, node, Wi, Wj):
    nterms = {"float32": 0, "bf16x2": 2, "bf16x3": 3}[BCAST_DT]
    seldt = np.float32 if nterms == 0 else __import__("ml_dtypes").bfloat16
    sel = np.zeros((CH, CH * NOUT), seldt)
    selz = np.zeros((CH, NOUT), np.float32)
    for p in range(CH):
        sel[p, NOUT * p : NOUT * (p + 1)] = 1.0
    selz[0, :] = 1.0
    wit = np.ascontiguousarray(Wi.T)
    wjt = np.ascontiguousarray(Wj.T)
    maps = []
    for c in range(NCORES):
        b, h = divmod(c, 2)
        r0 = RPC * h
        a = adj[b, 0, r0 : r0 + RPC, :]
        diag = np.zeros((CH, RPC), np.float32)
        diag[0, :] = a[np.arange(RPC), r0 + np.arange(RPC)]
        if h:
            ar = np.roll(a, -r0, axis=1)
            noder = np.roll(node[b], -r0, axis=1)
        else:
            ar = a
            noder = node[b]
        m = {
            "node_r": np.ascontiguousarray(noder),
            "diag16": diag,
            "wit": wit,
            "wjt": wjt,
            "sel": sel,
            "selz": selz,
        }
        for t, term in enumerate(_split_terms(ar.reshape(CH, FREE), nterms)):
            m[f"adj{t}"] = term
        maps.append(m)
    return maps


def kernel(**inputs):
    global last_results
    adj = np.asarray(inputs["adj"], dtype=np.float32)
    node = np.asarray(inputs["node"], dtype=np.float32)
    Wi = np.asarray(inputs["Wi"], dtype=np.float32)
    Wj = np.asarray(inputs["Wj"], dtype=np.float32)

    from concourse.bass_utils import run_bass_kernel_spmd

    nc = _build_nc()
    res = run_bass_kernel_spmd(nc, _in_maps(adj, node, Wi, Wj), list(range(NCORES)))
    last_results = res

    out = np.empty((B, NOUT, N, N), np.float32)
    for c in range(NCORES):
        b, h = divmod(c, 2)
        co = res.results[c]["out"].reshape(NOUT, RPC, N)
        if h:
            co = np.roll(co, RPC * h, axis=2)
        out[b, :, RPC * h : RPC * (h + 1), :] = co
    return out


# revision 8
# speedup vs baseline: 2.0553x; 1.2474x over previous
"""NodeConv kernel for 8 Trainium2 NeuronCores.

Reference computes, for adj [B,1,N,N], node [B,nin,N], Wi/Wj [nout,nin]:
    x  = node[:, :, None, :] * adj          # [B,nin,N,N]
    yi = einsum('oc,bcij->boij', Wi, x)
    yj = einsum('oc,bcij->boij', Wj, x)
    out = I * yi + (1-I) * yj

Because adj[b,i,j] does not depend on the contraction channel c, the
contraction factors out:
    off-diag: out[b,o,i,j] = adj[b,i,j] * (Wj @ node[b])[o,j]
    diag:     out[b,o,j,j] = adj[b,j,j] * (Wi @ node[b])[o,j]

So per batch we need two tiny matmuls (u = Wj@node, v = Wi@node) and a
broadcast multiply out[o,i,j] = adj[i,j]*u[o,j] with a diagonal patch.
The 128 MiB output write is the memory roofline.

Sharding: core c handles batch b=c//2, row half h=c%2 (128 rows). Odd
halves get their columns rolled by -128 on the host so the diagonal of
local row l sits at local column l on every core -> one SPMD program;
the host rolls the output back while gathering.

Per-core device program:
  - u = Wj @ node_r (PE), v = Wi @ node_r[:, :128] (PE),
    dv[o,l] = adj_diag[l] * v[o,l]
  - u_rep = u tiled 8x along the free dim -> [128, 2048]
  - per 8-row chunk p: PE broadcasts the 8 adj rows to all 128
    partitions with one-hot-selector matmuls.  adj is split on the host
    into NTERMS bf16 terms (2 terms ~2^-18 relative error, 3 exact)
    stacked along the contraction dim, so each 512-column slice is ONE
    bf16 matmul at K=16*NTERMS (the PSUM f32 accumulation sums the
    terms).  DVE multiplies by u_rep, ScalarE patches the 8 diagonal
    elements via a stride-257 view, and every SG chunks one DMA stores
    SG MiB, alternating between the two HWDGE rings.
"""

import os

import numpy as np

NCORES = 8
B, N, NIN, NOUT = 4, 256, 128, 128
RPC = 128          # rows per core
CH = 16            # chunks per core
RCH = 8            # rows per chunk
FREE = RCH * N     # 2048 free elems per chunk

NTERMS = int(os.environ.get("NODECONV_NTERMS", "2"))   # bf16 terms (2 or 3)
SG = int(os.environ.get("NODECONV_SG", "2"))           # chunks per store
OUT_BUFS = int(os.environ.get("NODECONV_OUT_BUFS", "3"))

KP = CH * NTERMS   # contraction partitions of the broadcast matmul

_cached = {}

last_results = None  # BassKernelResults of the most recent kernel() call


def _build_nc():
    key = (NTERMS, SG, OUT_BUFS)
    if key in _cached:
        return _cached[key]

    from contextlib import ExitStack

    import concourse.tile as tile
    from concourse import bacc, mybir

    f32 = mybir.dt.float32
    bf16 = mybir.dt.bfloat16

    nc = bacc.Bacc(
        "TRN2", target_bir_lowering=False, debug=False, num_devices=NCORES
    )

    # pk: [KP, 2*FREE] bf16 — adj terms in [:, :FREE], one-hot selector
    # blocks in [:, FREE:]
    pk = nc.dram_tensor("pk", [KP, 2 * FREE], bf16, kind="ExternalInput").ap()
    # ckf: [128, 512] f32 — node_r | WiT | WjT
    ckf = nc.dram_tensor("ckf", [NIN, N + 2 * NOUT], f32, kind="ExternalInput").ap()
    # dsz: [16, 256] f32 — diag row | f32 one-hot (selects partition 0)
    dsz = nc.dram_tensor("dsz", [CH, 2 * RPC], f32, kind="ExternalInput").ap()
    out = nc.dram_tensor("out", [NOUT, RPC * N], f32, kind="ExternalOutput").ap()

    with tile.TileContext(nc) as tc, ExitStack() as ctx:
        const = ctx.enter_context(tc.tile_pool(name="const", bufs=1))
        psum = ctx.enter_context(tc.tile_pool(name="psum", bufs=2, space="PSUM"))
        outp = ctx.enter_context(tc.tile_pool(name="outp", bufs=OUT_BUFS))

        pk_sb = const.tile([KP, 2 * FREE], bf16)
        nc.sync.dma_start(out=pk_sb[:], in_=pk)
        ckf_sb = const.tile([NIN, N + 2 * NOUT], f32)
        nc.sync.dma_start(out=ckf_sb[:], in_=ckf)
        dsz_sb = const.tile([CH, 2 * RPC], f32)
        nc.sync.dma_start(out=dsz_sb[:], in_=dsz)

        node_sb = ckf_sb[:, 0:N]
        wit_sb = ckf_sb[:, N : N + NOUT]
        wjt_sb = ckf_sb[:, N + NOUT : N + 2 * NOUT]
        diag_sb = dsz_sb[:, 0:RPC]
        selz_sb = dsz_sb[:, RPC : 2 * RPC]

        # u = Wj @ node_r  -> [nout, N]
        ps_u = psum.tile([NOUT, N], f32, tag="mm")
        nc.tensor.matmul(ps_u[:], lhsT=wjt_sb, rhs=node_sb, start=True, stop=True)
        u_sb = const.tile([NOUT, N], f32)
        nc.scalar.copy(u_sb[:], ps_u[:])

        # v = Wi @ node_r[:, :128]  (only the diagonal columns are needed)
        ps_v = psum.tile([NOUT, RPC], f32, tag="mm")
        nc.tensor.matmul(
            ps_v[:], lhsT=wit_sb, rhs=node_sb[:, 0:RPC], start=True, stop=True
        )
        v_sb = const.tile([NOUT, RPC], f32)
        nc.scalar.copy(v_sb[:], ps_v[:])

        # dv[o,l] = adj_diag[l] * v[o,l]; broadcast diag row over partitions
        ps_d = psum.tile([NOUT, RPC], f32, tag="mm")
        nc.tensor.matmul(ps_d[:], lhsT=selz_sb, rhs=diag_sb, start=True, stop=True)
        dv_sb = const.tile([NOUT, RPC], f32)
        nc.vector.tensor_mul(dv_sb[:], ps_d[:], v_sb[:])

        # u_rep: u tiled RCH times along the free dim
        u_rep = const.tile([NOUT, FREE], f32)
        nc.scalar.copy(u_rep[:, 0:N], u_sb[:])
        rep = N
        while rep < FREE:
            nc.scalar.copy(u_rep[:, rep : 2 * rep], u_rep[:, 0:rep])
            rep *= 2

        o_sb = None
        for p in range(CH):
            g = p % SG
            if g == 0:
                o_sb = outp.tile([NOUT, SG * FREE], f32, tag="osb")
            ps_b = psum.tile([NOUT, FREE], f32, tag="mm")
            lhs = pk_sb[:, FREE + NOUT * p : FREE + NOUT * (p + 1)]
            for q in range(FREE // 512):
                sl = slice(512 * q, 512 * (q + 1))
                nc.tensor.matmul(
                    ps_b[:, sl], lhsT=lhs, rhs=pk_sb[:, sl], start=True, stop=True
                )
            o_view = o_sb[:, g * FREE : (g + 1) * FREE]
            nc.vector.tensor_mul(o_view, ps_b[:], u_rep[:])
            # diagonal of local row l=8p+k sits at free offset 8p + k*257
            nc.scalar.copy(
                o_sb[
                    :,
                    g * FREE + RCH * p : g * FREE
                    + RCH * p
                    + (RCH - 1) * (N + 1)
                    + 1 : N + 1,
                ],
                dv_sb[:, RCH * p : RCH * (p + 1)],
            )
            if g == SG - 1:
                eng = nc.sync if (p // SG) % 2 == 0 else nc.scalar
                eng.dma_start(
                    out=out[:, FREE * (p - g) : FREE * (p + 1)], in_=o_sb[:]
                )

    nc.compile()
    _cached[key] = nc
    return nc


def _split_terms(x, nterms):
    """Split fp32 array into bf16 terms whose fp32 sum approximates x.
    2 terms leave <=2^-18 relative error; 3 terms are exact."""
    import ml_dtypes

    terms = []
    r = x
    for _ in range(nterms):
        t = r.astype(ml_dtypes.bfloat16)
        terms.append(t)
        r = (r - t.astype(np.float32)).astype(np.float32)
    return terms


def _in_maps(adj, node, Wi, Wj):
    import ml_dtypes

    bf16 = ml_dtypes.bfloat16
    sel = np.zeros((KP, CH * NOUT), bf16)
    for p in range(CH):
        for t in range(NTERMS):
            sel[CH * t + p, NOUT * p : NOUT * (p + 1)] = 1.0
    dszz = np.zeros((CH, 2 * RPC), np.float32)
    dszz[0, RPC : 2 * RPC] = 1.0
    ckf = np.empty((NIN, N + 2 * NOUT), np.float32)
    ckf[:, N : N + NOUT] = Wi.T
    ckf[:, N + NOUT :] = Wj.T
    maps = []
    for c in range(NCORES):
        b, h = divmod(c, 2)
        r0 = RPC * h
        a = adj[b, 0, r0 : r0 + RPC, :]
        dsz = dszz.copy()
        dsz[0, 0:RPC] = a[np.arange(RPC), r0 + np.arange(RPC)]
        if h:
            ar = np.roll(a, -r0, axis=1)
            noder = np.roll(node[b], -r0, axis=1)
        else:
            ar = a
            noder = node[b]
        pk = np.empty((KP, 2 * FREE), bf16)
        terms = _split_terms(ar.reshape(CH, FREE), NTERMS)
        for t in range(NTERMS):
            pk[CH * t : CH * (t + 1), 0:FREE] = terms[t]
        pk[:, FREE:] = sel
        m_ckf = ckf.copy()
        m_ckf[:, 0:N] = noder
        maps.append({"pk": pk, "ckf": m_ckf, "dsz": dsz})
    return maps


def kernel(**inputs):
    global last_results
    adj = np.asarray(inputs["adj"], dtype=np.float32)
    node = np.asarray(inputs["node"], dtype=np.float32)
    Wi = np.asarray(inputs["Wi"], dtype=np.float32)
    Wj = np.asarray(inputs["Wj"], dtype=np.float32)

    from concourse.bass_utils import run_bass_kernel_spmd

    nc = _build_nc()
    res = run_bass_kernel_spmd(nc, _in_maps(adj, node, Wi, Wj), list(range(NCORES)))
    last_results = res

    out = np.empty((B, NOUT, N, N), np.float32)
    for c in range(NCORES):
        b, h = divmod(c, 2)
        co = res.results[c]["out"].reshape(NOUT, RPC, N)
        if h:
            co = np.roll(co, RPC * h, axis=2)
        out[b, :, RPC * h : RPC * (h + 1), :] = co
    return out


# revision 11
# speedup vs baseline: 2.0871x; 1.0155x over previous
"""NodeConv kernel for 8 Trainium2 NeuronCores.

Reference computes, for adj [B,1,N,N], node [B,nin,N], Wi/Wj [nout,nin]:
    x  = node[:, :, None, :] * adj          # [B,nin,N,N]
    yi = einsum('oc,bcij->boij', Wi, x)
    yj = einsum('oc,bcij->boij', Wj, x)
    out = I * yi + (1-I) * yj

Because adj[b,i,j] does not depend on the contraction channel c, the
contraction factors out:
    off-diag: out[b,o,i,j] = adj[b,i,j] * (Wj @ node[b])[o,j]
    diag:     out[b,o,j,j] = adj[b,j,j] * (Wi @ node[b])[o,j]

So per batch we need two tiny matmuls (u = Wj@node, v = Wi@node) and a
broadcast multiply out[o,i,j] = adj[i,j]*u[o,j] with a diagonal patch.
The 128 MiB output write is the memory roofline.

Sharding: core c handles batch b=c//2, row half h=c%2 (128 rows). Odd
halves get their columns rolled by -128 on the host so the diagonal of
local row l sits at local column l on every core -> one SPMD program;
the host rolls the output back while gathering.

Per-core device program:
  - u = Wj @ node_r (PE), v = Wi @ node_r[:, :128] (PE),
    dv[o,l] = adj_diag[l] * v[o,l]
  - u_rep = u tiled 8x along the free dim -> [128, 2048]
  - per 8-row chunk p: PE broadcasts the 8 adj rows to all 128
    partitions with one-hot-selector matmuls.  adj is split on the host
    into NTERMS bf16 terms (2 terms ~2^-18 relative error, 3 exact)
    stacked along the contraction dim, so each 512-column slice is ONE
    bf16 matmul at K=16*NTERMS (the PSUM f32 accumulation sums the
    terms).  DVE multiplies by u_rep, ScalarE patches the 8 diagonal
    elements via a stride-257 view, and every SG chunks one DMA stores
    SG MiB, alternating between the two HWDGE rings.
"""

import os

import numpy as np

NCORES = 8
B, N, NIN, NOUT = 4, 256, 128, 128
RPC = 128          # rows per core
CH = 16            # chunks per core
RCH = 8            # rows per chunk
FREE = RCH * N     # 2048 free elems per chunk

NTERMS = int(os.environ.get("NODECONV_NTERMS", "2"))   # bf16 terms (2 or 3)
SG = int(os.environ.get("NODECONV_SG", "2"))           # chunks per store
OUT_BUFS = int(os.environ.get("NODECONV_OUT_BUFS", "3"))

KP = CH * NTERMS   # contraction partitions of the broadcast matmul

_cached = {}

last_results = None  # BassKernelResults of the most recent kernel() call


def _build_nc():
    key = (NTERMS, SG, OUT_BUFS)
    if key in _cached:
        return _cached[key]

    from contextlib import ExitStack

    import concourse.tile as tile
    from concourse import bacc, mybir

    f32 = mybir.dt.float32
    bf16 = mybir.dt.bfloat16

    nc = bacc.Bacc(
        "TRN2", target_bir_lowering=False, debug=False, num_devices=NCORES
    )

    # pk: [KP, 2*FREE] bf16 — adj terms in [:, :FREE], one-hot selector
    # blocks in [:, FREE:]
    pk = nc.dram_tensor("pk", [KP, 2 * FREE], bf16, kind="ExternalInput").ap()
    # ckf: [128, 512] f32 — node_r | WiT | WjT
    ckf = nc.dram_tensor("ckf", [NIN, N + 2 * NOUT], f32, kind="ExternalInput").ap()
    # dsz: [16, 256] f32 — diag row | f32 one-hot (selects partition 0)
    dsz = nc.dram_tensor("dsz", [CH, 2 * RPC], f32, kind="ExternalInput").ap()
    out = nc.dram_tensor("out", [NOUT, RPC * N], f32, kind="ExternalOutput").ap()

    with tile.TileContext(nc) as tc, ExitStack() as ctx:
        const = ctx.enter_context(tc.tile_pool(name="const", bufs=1))
        psum = ctx.enter_context(tc.tile_pool(name="psum", bufs=2, space="PSUM"))
        outp = ctx.enter_context(tc.tile_pool(name="outp", bufs=OUT_BUFS))

        # ckf first: the u matmul chain is the critical path to the first TT
        ckf_sb = const.tile([NIN, N + 2 * NOUT], f32)
        nc.sync.dma_start(out=ckf_sb[:], in_=ckf)
        pk_sb = const.tile([KP, 2 * FREE], bf16)
        nc.sync.dma_start(out=pk_sb[:], in_=pk)
        dsz_sb = const.tile([CH, 2 * RPC], f32)
        nc.scalar.dma_start(out=dsz_sb[:], in_=dsz)

        node_sb = ckf_sb[:, 0:N]
        wit_sb = ckf_sb[:, N : N + NOUT]
        wjt_sb = ckf_sb[:, N + NOUT : N + 2 * NOUT]
        diag_sb = dsz_sb[:, 0:RPC]
        selz_sb = dsz_sb[:, RPC : 2 * RPC]

        # u = Wj @ node_r  -> [nout, N]
        ps_u = psum.tile([NOUT, N], f32, tag="mm")
        nc.tensor.matmul(ps_u[:], lhsT=wjt_sb, rhs=node_sb, start=True, stop=True)
        u_sb = const.tile([NOUT, N], f32)
        nc.scalar.copy(u_sb[:], ps_u[:])

        # v = Wi @ node_r[:, :128]  (only the diagonal columns are needed)
        ps_v = psum.tile([NOUT, RPC], f32, tag="mm")
        nc.tensor.matmul(
            ps_v[:], lhsT=wit_sb, rhs=node_sb[:, 0:RPC], start=True, stop=True
        )
        v_sb = const.tile([NOUT, RPC], f32)
        nc.scalar.copy(v_sb[:], ps_v[:])

        # dv[o,l] = adj_diag[l] * v[o,l]; broadcast diag row over partitions
        ps_d = psum.tile([NOUT, RPC], f32, tag="mm")
        nc.tensor.matmul(ps_d[:], lhsT=selz_sb, rhs=diag_sb, start=True, stop=True)
        dv_sb = const.tile([NOUT, RPC], f32)
        nc.vector.tensor_mul(dv_sb[:], ps_d[:], v_sb[:])

        # u replicated RCH times along the free dim via a stride-0 view
        u_rep = u_sb[:].unsqueeze(1).broadcast_to([NOUT, RCH, N])

        o_sb = None
        for p in range(CH):
            g = p % SG
            if g == 0:
                o_sb = outp.tile([NOUT, SG * FREE], f32, tag="osb")
            ps_b = psum.tile([NOUT, FREE], f32, tag="mm")
            lhs = pk_sb[:, FREE + NOUT * p : FREE + NOUT * (p + 1)]
            for q in range(FREE // 512):
                sl = slice(512 * q, 512 * (q + 1))
                nc.tensor.matmul(
                    ps_b[:, sl], lhsT=lhs, rhs=pk_sb[:, sl], start=True, stop=True
                )
            o_view = o_sb[:, g * FREE : (g + 1) * FREE].rearrange(
                "p (k j) -> p k j", k=RCH
            )
            nc.vector.tensor_mul(o_view, ps_b[:].rearrange("p (k j) -> p k j", k=RCH), u_rep)
            # diagonal of local row l=8p+k sits at free offset 8p + k*257
            nc.scalar.copy(
                o_sb[
                    :,
                    g * FREE + RCH * p : g * FREE
                    + RCH * p
                    + (RCH - 1) * (N + 1)
                    + 1 : N + 1,
                ],
                dv_sb[:, RCH * p : RCH * (p + 1)],
            )
            if g == SG - 1:
                eng = nc.sync if (p // SG) % 2 == 0 else nc.scalar
                eng.dma_start(
                    out=out[:, FREE * (p - g) : FREE * (p + 1)], in_=o_sb[:]
                )

    nc.compile()
    _cached[key] = nc
    return nc


def _split_terms(x, nterms):
    """Split fp32 array into bf16 terms whose fp32 sum approximates x.
    2 terms leave <=2^-18 relative error; 3 terms are exact."""
    import ml_dtypes

    terms = []
    r = x
    for _ in range(nterms):
        t = r.astype(ml_dtypes.bfloat16)
        terms.append(t)
        r = (r - t.astype(np.float32)).astype(np.float32)
    return terms


def _in_maps(adj, node, Wi, Wj):
    import ml_dtypes

    bf16 = ml_dtypes.bfloat16
    sel = np.zeros((KP, CH * NOUT), bf16)
    for p in range(CH):
        for t in range(NTERMS):
            sel[CH * t + p, NOUT * p : NOUT * (p + 1)] = 1.0
    dszz = np.zeros((CH, 2 * RPC), np.float32)
    dszz[0, RPC : 2 * RPC] = 1.0
    ckf = np.empty((NIN, N + 2 * NOUT), np.float32)
    ckf[:, N : N + NOUT] = Wi.T
    ckf[:, N + NOUT :] = Wj.T
    maps = []
    for c in range(NCORES):
        b, h = divmod(c, 2)
        r0 = RPC * h
        a = adj[b, 0, r0 : r0 + RPC, :]
        dsz = dszz.copy()
        dsz[0, 0:RPC] = a[np.arange(RPC), r0 + np.arange(RPC)]
        if h:
            ar = np.roll(a, -r0, axis=1)
            noder = np.roll(node[b], -r0, axis=1)
        else:
            ar = a
            noder = node[b]
        pk = np.empty((KP, 2 * FREE), bf16)
        terms = _split_terms(ar.reshape(CH, FREE), NTERMS)
        for t in range(NTERMS):
            pk[CH * t : CH * (t + 1), 0:FREE] = terms[t]
        pk[:, FREE:] = sel
        m_ckf = ckf.copy()
        m_ckf[:, 0:N] = noder
        maps.append({"pk": pk, "ckf": m_ckf, "dsz": dsz})
    return maps


def kernel(**inputs):
    global last_results
    adj = np.asarray(inputs["adj"], dtype=np.float32)
    node = np.asarray(inputs["node"], dtype=np.float32)
    Wi = np.asarray(inputs["Wi"], dtype=np.float32)
    Wj = np.asarray(inputs["Wj"], dtype=np.float32)

    from concourse.bass_utils import run_bass_kernel_spmd

    nc = _build_nc()
    res = run_bass_kernel_spmd(nc, _in_maps(adj, node, Wi, Wj), list(range(NCORES)))
    last_results = res

    out = np.empty((B, NOUT, N, N), np.float32)
    for c in range(NCORES):
        b, h = divmod(c, 2)
        co = res.results[c]["out"].reshape(NOUT, RPC, N)
        if h:
            co = np.roll(co, RPC * h, axis=2)
        out[b, :, RPC * h : RPC * (h + 1), :] = co
    return out
